# revision 1
# baseline (speedup 1.0000x reference)
"""GQA self-attention kernel for Trainium2, sharded over 8 NeuronCores.

Problem: x[4, 2048, 1024], 16 heads / 4 KV groups / head_dim 64.
Sharding: batch (4) x head-half (2 KV groups each) = 8 cores.

Per-core dataflow (all "transposed world": features on partitions):
  xT[1024,2048] -> qT[512,2048], kT[128,2048], vT[128,2048] (PE matmuls, f32r)
  vT --PE transpose--> v_aug[seq,65] tiles (ones column appended for softmax sums)
  scores s[k,q] = kT_g^T(d,kpos) . qT_h(d,q)   (K=64 contraction, PE-array halves)
  p = exp(s/8)  (ACT engine, straight from PSUM)
  av[65,q] += v_aug^T p  (row 64 = softmax denominator)
  avT_norm = av[0:64] * recip(av[64])  (DVE; denominator row replicated via K=1 matmul)
  yT[e,q] = Wo_p^T . avT_norm  -> DRAM
Host: y[b] = (yT[2b] + yT[2b+1]).T + bo
"""

import sys
import numpy as np

sys.path.insert(0, "/opt/trn_rl_repo")

from contextlib import ExitStack

import concourse.bass as bass
import concourse.bacc as bacc
import concourse.mybir as mybir
from concourse import tile
from concourse.bass_utils import run_bass_kernel_spmd

F32 = mybir.dt.float32
F32R = mybir.dt.float32r

B, S, E = 4, 2048, 1024
NUM_HEADS, NUM_GROUPS, D = 16, 4, 64
CQ = 512          # q cols per core (8 heads)
CK = 128          # kv cols per core (2 groups)
ET = E // 128     # 8 embed K-tiles
SC = S // 512     # 4 seq chunks of 512
KT = S // 128     # 16 key tiles of 128
QT = CQ // 128    # 4 qT partition tiles
SCALE = 1.0 / np.sqrt(np.float32(D))

_NC_CACHE = {}


def build_nc():
    nc = bacc.Bacc(None, target_bir_lowering=False)

    xT = nc.dram_tensor("xT", [E, S], F32R, kind="ExternalInput")
    wq = nc.dram_tensor("wq", [E, CQ], F32R, kind="ExternalInput")
    wk = nc.dram_tensor("wk", [E, CK], F32R, kind="ExternalInput")
    wv = nc.dram_tensor("wv", [E, CK], F32R, kind="ExternalInput")
    wo = nc.dram_tensor("wo", [CQ, E], F32R, kind="ExternalInput")
    bqd = nc.dram_tensor("bqd", [128, QT], F32, kind="ExternalInput")
    bkd = nc.dram_tensor("bkd", [128, 1], F32, kind="ExternalInput")
    bvd = nc.dram_tensor("bvd", [128, 1], F32, kind="ExternalInput")
    identd = nc.dram_tensor("identd", [128, 128], F32R, kind="ExternalInput")
    onesd = nc.dram_tensor("onesd", [128, 64], F32R, kind="ExternalInput")
    yT = nc.dram_tensor("yT", [E, S], F32, kind="ExternalOutput")

    with tile.TileContext(nc) as tc, ExitStack() as ctx, \
            nc.allow_low_precision(reason="f32r is bit-identical to f32 here"):
        const = ctx.enter_context(tc.tile_pool(name="const", bufs=1))
        wpool = ctx.enter_context(tc.tile_pool(name="wpool", bufs=1))
        big = ctx.enter_context(tc.tile_pool(name="big", bufs=1))
        xpool = ctx.enter_context(tc.tile_pool(name="xpool", bufs=2))
        ppool = ctx.enter_context(tc.tile_pool(name="ppool", bufs=4))
        avpool = ctx.enter_context(tc.tile_pool(name="avpool", bufs=2))
        ypool = ctx.enter_context(tc.tile_pool(name="ypool", bufs=3))
        npool = ctx.enter_context(tc.tile_pool(name="npool", bufs=3))
        psA = ctx.enter_context(tc.tile_pool(name="psA", bufs=3, space="PSUM"))
        psAV = ctx.enter_context(tc.tile_pool(name="psAV", bufs=2, space="PSUM"))
        psY = ctx.enter_context(tc.tile_pool(name="psY", bufs=2, space="PSUM"))
        psM = ctx.enter_context(tc.tile_pool(name="psM", bufs=1, space="PSUM"))

        # ---- constants ----
        ident = const.tile([128, 128], F32R)
        nc.sync.dma_start(out=ident[:], in_=identd[:, :])
        ones_row = const.tile([1, 64], F32R)
        nc.sync.dma_start(out=ones_row[:], in_=onesd[0:1, :])

        # ---- weights + biases ----
        wq_sb = wpool.tile([128, ET, CQ], F32R)
        for et in range(ET):
            nc.sync.dma_start(out=wq_sb[:, et, :], in_=wq[et * 128:(et + 1) * 128, :])
        wk_sb = wpool.tile([128, ET, CK], F32R)
        wv_sb = wpool.tile([128, ET, CK], F32R)
        for et in range(ET):
            nc.sync.dma_start(out=wk_sb[:, et, :], in_=wk[et * 128:(et + 1) * 128, :])
            nc.sync.dma_start(out=wv_sb[:, et, :], in_=wv[et * 128:(et + 1) * 128, :])
        wo_sb = wpool.tile([128, QT, E], F32R)
        for t in range(QT):
            nc.sync.dma_start(out=wo_sb[:, t, :], in_=wo[t * 128:(t + 1) * 128, :])
        bq_sb = wpool.tile([128, QT], F32)
        nc.sync.dma_start(out=bq_sb[:], in_=bqd[:, :])
        bk_sb = wpool.tile([128, 1], F32)
        nc.sync.dma_start(out=bk_sb[:], in_=bkd[:, :])
        bv_sb = wpool.tile([128, 1], F32)
        nc.sync.dma_start(out=bv_sb[:], in_=bvd[:, :])

        # ---- persistent activations ----
        qT_sb = big.tile([128, QT, S], F32R)      # 32KB/partition
        kT_sb = big.tile([128, S], F32R)          # 8KB
        vT_sb = big.tile([128, S], F32R)          # 8KB
        vaug = big.tile([128, 2 * KT, 65], F32R)  # v natural + ones col, per (g, kt)
        for g in range(2):
            for kt in range(KT):
                nc.sync.dma_start(out=vaug[:, g * KT + kt, 64:65],
                                  in_=onesd[:, 0:1])

        ADD = mybir.AluOpType.add
        MUL = mybir.AluOpType.mult

        # ================= phase 1: projections =================
        for sc in range(SC):
            lo = sc * 512
            xt = xpool.tile([128, ET, 512], F32R, tag="xt", name=f"xt{sc}")
            for et in range(ET):
                nc.sync.dma_start(
                    out=xt[:, et, :], in_=xT[et * 128:(et + 1) * 128, lo:lo + 512])
            # qT
            for t in range(QT):
                pq = psA.tile([128, 512], F32, tag="mm", name=f"pq{sc}_{t}")
                for et in range(ET):
                    nc.tensor.matmul(
                        pq[:], wq_sb[:, et, t * 128:(t + 1) * 128],
                        xt[:, et, :], start=(et == 0), stop=(et == ET - 1))
                nc.vector.tensor_scalar(
                    out=qT_sb[:, t, lo:lo + 512], in0=pq[:],
                    scalar1=bq_sb[:, t:t + 1], scalar2=None, op0=ADD)
            # kT / vT
            pk = psA.tile([128, 512], F32, tag="mm", name=f"pk{sc}")
            for et in range(ET):
                nc.tensor.matmul(pk[:], wk_sb[:, et, :], xt[:, et, :],
                                 start=(et == 0), stop=(et == ET - 1))
            nc.vector.tensor_scalar(out=kT_sb[:, lo:lo + 512], in0=pk[:],
                                    scalar1=bk_sb[:, 0:1], scalar2=None, op0=ADD)
            pv = psA.tile([128, 512], F32, tag="mm", name=f"pv{sc}")
            for et in range(ET):
                nc.tensor.matmul(pv[:], wv_sb[:, et, :], xt[:, et, :],
                                 start=(et == 0), stop=(et == ET - 1))
            nc.vector.tensor_scalar(out=vT_sb[:, lo:lo + 512], in0=pv[:],
                                    scalar1=bv_sb[:, 0:1], scalar2=None, op0=ADD)
            # transpose vT chunk -> v natural tiles (with ones col kept intact)
            for ktl in range(4):
                kt = sc * 4 + ktl
                ptr = psM.tile([128, 128], F32R, tag="misc", name=f"ptr{kt}")
                nc.tensor.transpose(ptr[:], vT_sb[:, kt * 128:(kt + 1) * 128], ident[:])
                for g in range(2):
                    nc.vector.tensor_copy(
                        out=vaug[:, g * KT + kt, 0:64], in_=ptr[:, g * 64:(g + 1) * 64])

        # ================= phase 2: attention + out-proj =================
        for qc in range(SC):
            lo = qc * 512
            avT = avpool.tile([128, QT, 512], F32R, tag="avT", name=f"avT{qc}")
            for h in range(8):
                t, g = h % 4, h // 4
                ph = g * 64
                avp = psAV.tile([128, 512], F32, tag="av", name=f"avp{qc}_{h}")
                for kt in range(KT):
                    sp = psA.tile([128, 512], F32, tag="mm", name=f"sp{qc}_{h}_{kt}")
                    nc.tensor.matmul(
                        sp[:],
                        kT_sb[ph:ph + 64, kt * 128:(kt + 1) * 128],
                        qT_sb[ph:ph + 64, t, lo:lo + 512],
                        start=True, stop=True)
                    pe = ppool.tile([128, 512], F32R, tag="pexp", name=f"pe{qc}_{h}_{kt}")
                    nc.scalar.activation(
                        pe[:], sp[:], mybir.ActivationFunctionType.Exp,
                        scale=float(SCALE))
                    nc.tensor.matmul(
                        avp[0:65, :], vaug[:, g * KT + kt, :], pe[:],
                        start=(kt == 0), stop=(kt == KT - 1))
                # normalize: avT_norm = av[0:64] * (1 / av[64])
                linv = npool.tile([1, 512], F32R, tag="linv", name=f"linv{qc}_{h}")
                nc.vector.reciprocal(linv[:], avp[64:65, :])
                lrp = psM.tile([128, 512], F32, tag="misc", name=f"lrp{qc}_{h}")
                nc.tensor.matmul(lrp[0:64, :], ones_row[:], linv[:],
                                 start=True, stop=True)
                lrep = npool.tile([64, 512], F32, tag="lrep", name=f"lrep{qc}_{h}")
                nc.vector.tensor_copy(out=lrep[:], in_=lrp[0:64, :])
                nc.vector.tensor_tensor(
                    out=avT[ph:ph + 64, t, :], in0=avp[0:64, :], in1=lrep[:], op=MUL)
            # out projection for this q chunk
            for et in range(ET):
                yp = psY.tile([128, 512], F32, tag="y", name=f"yp{qc}_{et}")
                for t in range(QT):
                    nc.tensor.matmul(
                        yp[:], wo_sb[:, t, et * 128:(et + 1) * 128],
                        avT[:, t, :], start=(t == 0), stop=(t == QT - 1))
                ysb = ypool.tile([128, 512], F32, tag="ysb", name=f"ysb{qc}_{et}")
                nc.vector.tensor_copy(out=ysb[:], in_=yp[:])
                nc.sync.dma_start(out=yT[et * 128:(et + 1) * 128, lo:lo + 512],
                                  in_=ysb[:])
    nc.compile()
    return nc


def _shard_inputs(x, Wq, bq, Wk, bk, Wv, bv, Wo, bo):
    """Build the 8 per-core input maps."""
    x = np.asarray(x, dtype=np.float32)
    in_maps = []
    for c in range(8):
        b, H = c // 2, c % 2
        heads = [8 * H + t for t in range(4)] + [8 * H + t + 4 for t in range(4)]
        # qT tile t holds (local head t -> partitions 0-63, local head t+4 -> 64-127)
        order = []
        for t in range(4):
            order.extend(range(heads[t] * 64, heads[t] * 64 + 64))
            order.extend(range(heads[t + 4] * 64, heads[t + 4] * 64 + 64))
        order = np.asarray(order)
        wq_p = np.ascontiguousarray(np.asarray(Wq, np.float32)[:, order])
        bq_p = np.ascontiguousarray(
            np.asarray(bq, np.float32)[order].reshape(4, 128).T)
        wo_p = np.ascontiguousarray(np.asarray(Wo, np.float32)[order, :])
        wk_s = np.ascontiguousarray(np.asarray(Wk, np.float32)[:, H * 128:(H + 1) * 128])
        wv_s = np.ascontiguousarray(np.asarray(Wv, np.float32)[:, H * 128:(H + 1) * 128])
        bk_s = np.ascontiguousarray(np.asarray(bk, np.float32)[H * 128:(H + 1) * 128]
                                    .reshape(128, 1))
        bv_s = np.ascontiguousarray(np.asarray(bv, np.float32)[H * 128:(H + 1) * 128]
                                    .reshape(128, 1))
        xT_b = np.ascontiguousarray(x[b].T)
        in_maps.append({
            "xT": xT_b, "wq": wq_p, "wk": wk_s, "wv": wv_s, "wo": wo_p,
            "bqd": bq_p, "bkd": bk_s, "bvd": bv_s,
            "identd": np.eye(128, dtype=np.float32),
            "onesd": np.ones((128, 64), dtype=np.float32),
        })
    return in_maps


def kernel(x, Wq, bq, Wk, bk, Wv, bv, Wo, bo, _trace=False):
    if "nc" not in _NC_CACHE:
        _NC_CACHE["nc"] = build_nc()
    nc = _NC_CACHE["nc"]
    in_maps = _shard_inputs(x, Wq, bq, Wk, bk, Wv, bv, Wo, bo)
    res = run_bass_kernel_spmd(nc, in_maps, list(range(8)), trace=_trace)
    bo = np.asarray(bo, dtype=np.float32)
    out = np.empty((B, S, E), dtype=np.float32)
    for b in range(B):
        yT = res.results[2 * b]["yT"] + res.results[2 * b + 1]["yT"]
        out[b] = yT.T + bo
    if _trace:
        return out, res
    return out



# revision 5
# speedup vs baseline: 1.8461x; 1.8461x over previous
"""GQA self-attention kernel for Trainium2, sharded over 8 NeuronCores.

Problem: x[4, 2048, 1024], 16 heads / 4 KV groups / head_dim 64.
Sharding: batch (4) x head-half (2 KV groups each) = 8 cores.

All matmuls run in bf16 (PE 1 cycle/row; fp32/f32r ran at fp32_mode=HIGH,
~3-4 cycles/row on this hardware). PSUM accumulation stays fp32.

Per-core dataflow (all "transposed world": features on partitions):
  xT[1024,2048] -> qT[512,2048], kT[128,2048], vT[128,2048] (PE matmuls)
  vT --PE transpose--> v_aug[seq,65] tiles (ones column appended)
  scores s[k,q] = kT_g^T(d,kpos) . qT_h(d,q)   (K=64 contraction)
  p = exp(s/8)  (ACT engine, PSUM f32 -> SBUF bf16)
  av[65,q] += v_aug^T p  (row 64 = softmax denominator)
  avT_norm = av[0:64] * recip_fast(av[64]) broadcast  (DVE + PE K=1 matmul)
  yT[e,q] = Wo_p^T . avT_norm  -> DRAM (f32)
Host: y[b] = (yT[2b] + yT[2b+1]).T + bo
"""

import sys
import numpy as np
import ml_dtypes

sys.path.insert(0, "/opt/trn_rl_repo")

from contextlib import ExitStack

import concourse.bass as bass
import concourse.bacc as bacc
import concourse.mybir as mybir
from concourse import tile
from concourse.bass_utils import run_bass_kernel_spmd

F32 = mybir.dt.float32
BF16 = mybir.dt.bfloat16
BF16_NP = ml_dtypes.bfloat16

B, S, E = 4, 2048, 1024
NUM_HEADS, NUM_GROUPS, D = 16, 4, 64
CQ = 512          # q cols per core (8 heads)
CK = 128          # kv cols per core (2 groups)
ET = E // 128     # 8 embed K-tiles
SC = S // 512     # 4 seq chunks of 512
KT = S // 128     # 16 key tiles of 128
QT = CQ // 128    # 4 qT partition tiles
SCALE = 1.0 / np.sqrt(np.float32(D))

_NC_CACHE = {}


def build_nc():
    nc = bacc.Bacc(None, target_bir_lowering=False)

    xT = nc.dram_tensor("xT", [E, S], BF16, kind="ExternalInput")
    wq = nc.dram_tensor("wq", [E, CQ], BF16, kind="ExternalInput")
    wk = nc.dram_tensor("wk", [E, CK], BF16, kind="ExternalInput")
    wv = nc.dram_tensor("wv", [E, CK], BF16, kind="ExternalInput")
    wo = nc.dram_tensor("wo", [CQ, E], BF16, kind="ExternalInput")
    bqd = nc.dram_tensor("bqd", [128, QT], F32, kind="ExternalInput")
    bkd = nc.dram_tensor("bkd", [128, 1], F32, kind="ExternalInput")
    bvd = nc.dram_tensor("bvd", [128, 1], F32, kind="ExternalInput")
    identd = nc.dram_tensor("identd", [128, 128], BF16, kind="ExternalInput")
    onesd = nc.dram_tensor("onesd", [128, 64], BF16, kind="ExternalInput")
    yT = nc.dram_tensor("yT", [E, S], F32, kind="ExternalOutput")

    with tile.TileContext(nc) as tc, ExitStack() as ctx, \
            nc.allow_low_precision(reason="bf16 matmuls; rel-err budget 2e-2"):
        const = ctx.enter_context(tc.tile_pool(name="const", bufs=1))
        wpool = ctx.enter_context(tc.tile_pool(name="wpool", bufs=1))
        big = ctx.enter_context(tc.tile_pool(name="big", bufs=1))
        xpool = ctx.enter_context(tc.tile_pool(name="xpool", bufs=2))
        ppool = ctx.enter_context(tc.tile_pool(name="ppool", bufs=4))
        avpool = ctx.enter_context(tc.tile_pool(name="avpool", bufs=2))
        ypool = ctx.enter_context(tc.tile_pool(name="ypool", bufs=3))
        npool = ctx.enter_context(tc.tile_pool(name="npool", bufs=3))
        psA = ctx.enter_context(tc.tile_pool(name="psA", bufs=3, space="PSUM"))
        psAV = ctx.enter_context(tc.tile_pool(name="psAV", bufs=2, space="PSUM"))
        psY = ctx.enter_context(tc.tile_pool(name="psY", bufs=2, space="PSUM"))
        psM = ctx.enter_context(tc.tile_pool(name="psM", bufs=1, space="PSUM"))

        # ---- constants ----
        ident = const.tile([128, 128], BF16)
        nc.sync.dma_start(out=ident[:], in_=identd[:, :])
        ones_row = const.tile([1, 64], BF16)
        nc.sync.dma_start(out=ones_row[:], in_=onesd[0:1, :])

        # ---- weights + biases ----
        wq_sb = wpool.tile([128, ET, CQ], BF16)
        for et in range(ET):
            nc.sync.dma_start(out=wq_sb[:, et, :], in_=wq[et * 128:(et + 1) * 128, :])
        wk_sb = wpool.tile([128, ET, CK], BF16)
        wv_sb = wpool.tile([128, ET, CK], BF16)
        for et in range(ET):
            nc.sync.dma_start(out=wk_sb[:, et, :], in_=wk[et * 128:(et + 1) * 128, :])
            nc.sync.dma_start(out=wv_sb[:, et, :], in_=wv[et * 128:(et + 1) * 128, :])
        wo_sb = wpool.tile([128, QT, E], BF16)
        for t in range(QT):
            nc.sync.dma_start(out=wo_sb[:, t, :], in_=wo[t * 128:(t + 1) * 128, :])
        bq_sb = wpool.tile([128, QT], F32)
        nc.sync.dma_start(out=bq_sb[:], in_=bqd[:, :])
        bk_sb = wpool.tile([128, 1], F32)
        nc.sync.dma_start(out=bk_sb[:], in_=bkd[:, :])
        bv_sb = wpool.tile([128, 1], F32)
        nc.sync.dma_start(out=bv_sb[:], in_=bvd[:, :])

        # ---- persistent activations ----
        qT_sb = big.tile([128, QT, S], BF16)      # 16KB/partition
        kT_sb = big.tile([128, S], BF16)          # 4KB
        vT_sb = big.tile([128, S], BF16)          # 4KB
        vaug = big.tile([128, 2 * KT, 65], BF16)  # v natural + ones col, per (g, kt)
        for g in range(2):
            for kt in range(KT):
                nc.sync.dma_start(out=vaug[:, g * KT + kt, 64:65],
                                  in_=onesd[:, 0:1])

        ADD = mybir.AluOpType.add
        MUL = mybir.AluOpType.mult

        # ================= phase 1: projections =================
        for sc in range(SC):
            lo = sc * 512
            xt = xpool.tile([128, ET, 512], BF16, tag="xt", name=f"xt{sc}")
            for et in range(ET):
                nc.sync.dma_start(
                    out=xt[:, et, :], in_=xT[et * 128:(et + 1) * 128, lo:lo + 512])
            # qT
            for t in range(QT):
                pq = psA.tile([128, 512], F32, tag="mm", name=f"pq{sc}_{t}")
                for et in range(ET):
                    nc.tensor.matmul(
                        pq[:], wq_sb[:, et, t * 128:(t + 1) * 128],
                        xt[:, et, :], start=(et == 0), stop=(et == ET - 1))
                nc.vector.tensor_scalar(
                    out=qT_sb[:, t, lo:lo + 512], in0=pq[:],
                    scalar1=bq_sb[:, t:t + 1], scalar2=None, op0=ADD)
            # kT / vT
            pk = psA.tile([128, 512], F32, tag="mm", name=f"pk{sc}")
            for et in range(ET):
                nc.tensor.matmul(pk[:], wk_sb[:, et, :], xt[:, et, :],
                                 start=(et == 0), stop=(et == ET - 1))
            nc.vector.tensor_scalar(out=kT_sb[:, lo:lo + 512], in0=pk[:],
                                    scalar1=bk_sb[:, 0:1], scalar2=None, op0=ADD)
            pv = psA.tile([128, 512], F32, tag="mm", name=f"pv{sc}")
            for et in range(ET):
                nc.tensor.matmul(pv[:], wv_sb[:, et, :], xt[:, et, :],
                                 start=(et == 0), stop=(et == ET - 1))
            nc.vector.tensor_scalar(out=vT_sb[:, lo:lo + 512], in0=pv[:],
                                    scalar1=bv_sb[:, 0:1], scalar2=None, op0=ADD)
            # transpose vT chunk -> v natural tiles (with ones col kept intact)
            for ktl in range(4):
                kt = sc * 4 + ktl
                ptr = psM.tile([128, 128], BF16, tag="misc", name=f"ptr{kt}")
                nc.tensor.transpose(ptr[:], vT_sb[:, kt * 128:(kt + 1) * 128], ident[:])
                for g in range(2):
                    nc.vector.tensor_copy(
                        out=vaug[:, g * KT + kt, 0:64], in_=ptr[:, g * 64:(g + 1) * 64])

        # ================= phase 2: attention + out-proj =================
        for qc in range(SC):
            lo = qc * 512
            avT = avpool.tile([128, QT, 512], BF16, tag="avT", name=f"avT{qc}")
            for h in range(8):
                t, g = h % 4, h // 4
                ph = g * 64
                avp = psAV.tile([128, 512], F32, tag="av", name=f"avp{qc}_{h}")
                for kt in range(KT):
                    sp = psA.tile([128, 512], F32, tag="mm", name=f"sp{qc}_{h}_{kt}")
                    nc.tensor.matmul(
                        sp[:],
                        kT_sb[ph:ph + 64, kt * 128:(kt + 1) * 128],
                        qT_sb[ph:ph + 64, t, lo:lo + 512],
                        start=True, stop=True)
                    pe = ppool.tile([128, 512], BF16, tag="pexp", name=f"pe{qc}_{h}_{kt}")
                    nc.scalar.activation(
                        pe[:], sp[:], mybir.ActivationFunctionType.Exp,
                        scale=float(SCALE))
                    nc.tensor.matmul(
                        avp[0:65, :], vaug[:, g * KT + kt, :], pe[:],
                        start=(kt == 0), stop=(kt == KT - 1))
                # normalize: avT_norm = av[0:64] * (1 / av[64])
                den = npool.tile([1, 512], F32, tag="den", name=f"den{qc}_{h}")
                nc.vector.tensor_copy(out=den[:], in_=avp[64:65, :])
                linv = npool.tile([1, 512], F32, tag="linv", name=f"linv{qc}_{h}")
                nc.vector.reciprocal_approx_fast(out=linv[:], in_=den[:])
                linb = npool.tile([1, 512], BF16, tag="linb", name=f"linb{qc}_{h}")
                nc.vector.tensor_copy(out=linb[:], in_=linv[:])
                lrp = psM.tile([128, 512], F32, tag="misc", name=f"lrp{qc}_{h}")
                nc.tensor.matmul(lrp[0:64, :], ones_row[:], linb[:],
                                 start=True, stop=True)
                lrep = npool.tile([64, 512], F32, tag="lrep", name=f"lrep{qc}_{h}")
                nc.vector.tensor_copy(out=lrep[:], in_=lrp[0:64, :])
                nc.vector.tensor_tensor(
                    out=avT[ph:ph + 64, t, :], in0=avp[0:64, :], in1=lrep[:],
                    op=MUL)
            # out projection for this q chunk
            for et in range(ET):
                yp = psY.tile([128, 512], F32, tag="y", name=f"yp{qc}_{et}")
                for t in range(QT):
                    nc.tensor.matmul(
                        yp[:], wo_sb[:, t, et * 128:(et + 1) * 128],
                        avT[:, t, :], start=(t == 0), stop=(t == QT - 1))
                ysb = ypool.tile([128, 512], F32, tag="ysb", name=f"ysb{qc}_{et}")
                nc.vector.tensor_copy(out=ysb[:], in_=yp[:])
                nc.sync.dma_start(out=yT[et * 128:(et + 1) * 128, lo:lo + 512],
                                  in_=ysb[:])
    nc.compile()
    return nc


def _shard_inputs(x, Wq, bq, Wk, bk, Wv, bv, Wo, bo):
    """Build the 8 per-core input maps."""
    x = np.asarray(x, dtype=np.float32)
    in_maps = []
    for c in range(8):
        b, H = c // 2, c % 2
        heads = [8 * H + t for t in range(4)] + [8 * H + t + 4 for t in range(4)]
        # qT tile t holds (local head t -> partitions 0-63, local head t+4 -> 64-127)
        order = []
        for t in range(4):
            order.extend(range(heads[t] * 64, heads[t] * 64 + 64))
            order.extend(range(heads[t + 4] * 64, heads[t + 4] * 64 + 64))
        order = np.asarray(order)
        wq_p = np.ascontiguousarray(np.asarray(Wq, np.float32)[:, order]).astype(BF16_NP)
        bq_p = np.ascontiguousarray(
            np.asarray(bq, np.float32)[order].reshape(4, 128).T)
        wo_p = np.ascontiguousarray(np.asarray(Wo, np.float32)[order, :]).astype(BF16_NP)
        wk_s = np.ascontiguousarray(
            np.asarray(Wk, np.float32)[:, H * 128:(H + 1) * 128]).astype(BF16_NP)
        wv_s = np.ascontiguousarray(
            np.asarray(Wv, np.float32)[:, H * 128:(H + 1) * 128]).astype(BF16_NP)
        bk_s = np.ascontiguousarray(np.asarray(bk, np.float32)[H * 128:(H + 1) * 128]
                                    .reshape(128, 1))
        bv_s = np.ascontiguousarray(np.asarray(bv, np.float32)[H * 128:(H + 1) * 128]
                                    .reshape(128, 1))
        xT_b = np.ascontiguousarray(x[b].T).astype(BF16_NP)
        in_maps.append({
            "xT": xT_b, "wq": wq_p, "wk": wk_s, "wv": wv_s, "wo": wo_p,
            "bqd": bq_p, "bkd": bk_s, "bvd": bv_s,
            "identd": np.eye(128, dtype=BF16_NP),
            "onesd": np.ones((128, 64), dtype=BF16_NP),
        })
    return in_maps


def kernel(x, Wq, bq, Wk, bk, Wv, bv, Wo, bo, _trace=False):
    if "nc" not in _NC_CACHE:
        _NC_CACHE["nc"] = build_nc()
    nc = _NC_CACHE["nc"]
    in_maps = _shard_inputs(x, Wq, bq, Wk, bk, Wv, bv, Wo, bo)
    res = run_bass_kernel_spmd(nc, in_maps, list(range(8)), trace=_trace)
    bo = np.asarray(bo, dtype=np.float32)
    out = np.empty((B, S, E), dtype=np.float32)
    for b in range(B):
        yT = res.results[2 * b]["yT"] + res.results[2 * b + 1]["yT"]
        out[b] = yT.T + bo
    if _trace:
        return out, res
    return out


# revision 8
# speedup vs baseline: 1.9001x; 1.0293x over previous
"""GQA self-attention kernel for Trainium2, sharded over 8 NeuronCores.

Problem: x[4, 2048, 1024], 16 heads / 4 KV groups / head_dim 64.
Sharding: batch (4) x head-half (2 KV groups each) = 8 cores.

All matmuls run in bf16 (PE 1 cycle/col; fp32 runs multi-pass). PSUM
accumulation stays fp32. exp runs on ACT over 1024-wide tiles spanning two
PSUM banks to amortize the per-instruction access latency. DMA issues are
spread across engine queues (sync/scalar/vector/gpsimd) because descriptor
generation (~0.6us per dma_start) serializes on a single engine otherwise.

Per-core dataflow (all "transposed world": features on partitions):
  xT[1024,2048] -> qT[512,2048], kT[128,2048], vT[128,2048] (PE matmuls)
  vT --PE transpose--> v_aug[seq,65] tiles (ones column appended)
  scores s[k,q] = kT_g^T(d,kpos) . qT_h(d,q)   (K=64 contraction)
  p = exp(s/8)  (ACT engine, PSUM f32 -> SBUF bf16, 1024 cols/instr)
  av[65,q] += v_aug^T p  (row 64 = softmax denominator)
  avT_norm = av[0:64] * recip_fast(av[64]) bcast (DVE + GpSimd broadcast)
  yT[e,q] = Wo_p^T . avT_norm  -> DRAM (f32)
Host: y[b] = (yT[2b] + yT[2b+1]).T + bo
"""

import sys
import numpy as np
import ml_dtypes

sys.path.insert(0, "/opt/trn_rl_repo")

from contextlib import ExitStack

import concourse.bass as bass
import concourse.bacc as bacc
import concourse.mybir as mybir
from concourse import tile
from concourse.bass_utils import run_bass_kernel_spmd

F32 = mybir.dt.float32
BF16 = mybir.dt.bfloat16
BF16_NP = ml_dtypes.bfloat16

B, S, E = 4, 2048, 1024
NUM_HEADS, NUM_GROUPS, D = 16, 4, 64
CQ = 512          # q cols per core (8 heads)
CK = 128          # kv cols per core (2 groups)
ET = E // 128     # 8 embed K-tiles
SC = S // 512     # 4 seq chunks of 512
KT = S // 128     # 16 key tiles of 128
QT = CQ // 128    # 4 qT partition tiles
SCALE = 1.0 / np.sqrt(np.float32(D))

_NC_CACHE = {}


def build_nc():
    nc = bacc.Bacc(None, target_bir_lowering=False)

    # DRAM layouts are partition-major ([128, blocks, cols]) so each logical
    # load/store is a single dma_start.
    xT = nc.dram_tensor("xT", [128, ET, S], BF16, kind="ExternalInput")
    wq = nc.dram_tensor("wq", [128, ET, CQ], BF16, kind="ExternalInput")
    wk = nc.dram_tensor("wk", [128, ET, CK], BF16, kind="ExternalInput")
    wv = nc.dram_tensor("wv", [128, ET, CK], BF16, kind="ExternalInput")
    wo = nc.dram_tensor("wo", [128, QT, E], BF16, kind="ExternalInput")
    bqd = nc.dram_tensor("bqd", [128, QT], F32, kind="ExternalInput")
    bkd = nc.dram_tensor("bkd", [128, 1], F32, kind="ExternalInput")
    bvd = nc.dram_tensor("bvd", [128, 1], F32, kind="ExternalInput")
    identd = nc.dram_tensor("identd", [128, 128], BF16, kind="ExternalInput")
    onesd = nc.dram_tensor("onesd", [128, 2 * KT], BF16, kind="ExternalInput")
    yT = nc.dram_tensor("yT", [128, ET, S], F32, kind="ExternalOutput")

    with tile.TileContext(nc) as tc, ExitStack() as ctx, \
            nc.allow_low_precision(reason="bf16 matmuls; rel-err budget 2e-2"):
        const = ctx.enter_context(tc.tile_pool(name="const", bufs=1))
        wpool = ctx.enter_context(tc.tile_pool(name="wpool", bufs=1))
        big = ctx.enter_context(tc.tile_pool(name="big", bufs=1))
        xpool = ctx.enter_context(tc.tile_pool(name="xpool", bufs=2))
        ppool = ctx.enter_context(tc.tile_pool(name="ppool", bufs=4))
        avpool = ctx.enter_context(tc.tile_pool(name="avpool", bufs=2))
        npool = ctx.enter_context(tc.tile_pool(name="npool", bufs=3))
        ypool = ctx.enter_context(tc.tile_pool(name="ypool", bufs=3))
        psA = ctx.enter_context(tc.tile_pool(name="psA", bufs=2, space="PSUM"))
        psAV = ctx.enter_context(tc.tile_pool(name="psAV", bufs=2, space="PSUM"))
        psY = ctx.enter_context(tc.tile_pool(name="psY", bufs=2, space="PSUM"))

        # ---- constants ----
        ident = const.tile([128, 128], BF16)
        nc.scalar.dma_start(out=ident[:], in_=identd[:, :])

        # ---- weights + biases (issue across engines to parallelize DGE) ----
        wq_sb = wpool.tile([128, ET, CQ], BF16)
        nc.sync.dma_start(out=wq_sb[:], in_=wq[:, :, :])
        wk_sb = wpool.tile([128, ET, CK], BF16)
        nc.gpsimd.dma_start(out=wk_sb[:], in_=wk[:, :, :])
        wv_sb = wpool.tile([128, ET, CK], BF16)
        nc.gpsimd.dma_start(out=wv_sb[:], in_=wv[:, :, :])
        wo_sb = wpool.tile([128, QT, E], BF16)
        nc.gpsimd.dma_start(out=wo_sb[:], in_=wo[:, :, :])
        bq_sb = wpool.tile([128, QT], F32)
        nc.scalar.dma_start(out=bq_sb[:], in_=bqd[:, :])
        bk_sb = wpool.tile([128, 1], F32)
        nc.scalar.dma_start(out=bk_sb[:], in_=bkd[:, :])
        bv_sb = wpool.tile([128, 1], F32)
        nc.scalar.dma_start(out=bv_sb[:], in_=bvd[:, :])

        # ---- persistent activations ----
        qT_sb = big.tile([128, QT, S], BF16)      # 16KB/partition
        kT_sb = big.tile([128, S], BF16)          # 4KB
        vT_sb = big.tile([128, S], BF16)          # 4KB
        # v natural + ones row, laid out [part, 65, tile] so the ones row
        # loads in ONE dma  (tile idx = g*KT+kt; stationary AP vaug[:, :, i])
        vaug = big.tile([128, 65, 2 * KT], BF16)
        nc.scalar.dma_start(out=vaug[:, 64, :], in_=onesd[:, :])

        ADD = mybir.AluOpType.add
        MUL = mybir.AluOpType.mult

        # ================= phase 1: projections =================
        for sc in range(SC):
            lo = sc * 512
            xt = xpool.tile([128, ET, 512], BF16, tag="xt", name=f"xt{sc}")
            # spread the 8 block loads over 4 engine queues
            issuers = [nc.sync, nc.scalar, nc.gpsimd]
            for et in range(ET):
                issuers[et % 3].dma_start(
                    out=xt[:, et, :], in_=xT[:, et, lo:lo + 512])
            # qT
            for t in range(QT):
                pq = psA.tile([128, 1024], F32, tag="mm", name=f"pq{sc}_{t}")
                for et in range(ET):
                    nc.tensor.matmul(
                        pq[:, 0:512], wq_sb[:, et, t * 128:(t + 1) * 128],
                        xt[:, et, :], start=(et == 0), stop=(et == ET - 1))
                nc.vector.tensor_scalar(
                    out=qT_sb[:, t, lo:lo + 512], in0=pq[:, 0:512],
                    scalar1=bq_sb[:, t:t + 1], scalar2=None, op0=ADD)
            # kT / vT
            pk = psA.tile([128, 1024], F32, tag="mm", name=f"pk{sc}")
            for et in range(ET):
                nc.tensor.matmul(pk[:, 0:512], wk_sb[:, et, :], xt[:, et, :],
                                 start=(et == 0), stop=(et == ET - 1))
            nc.vector.tensor_scalar(out=kT_sb[:, lo:lo + 512], in0=pk[:, 0:512],
                                    scalar1=bk_sb[:, 0:1], scalar2=None, op0=ADD)
            pv = psA.tile([128, 1024], F32, tag="mm", name=f"pv{sc}")
            for et in range(ET):
                nc.tensor.matmul(pv[:, 0:512], wv_sb[:, et, :], xt[:, et, :],
                                 start=(et == 0), stop=(et == ET - 1))
            nc.vector.tensor_scalar(out=vT_sb[:, lo:lo + 512], in0=pv[:, 0:512],
                                    scalar1=bv_sb[:, 0:1], scalar2=None, op0=ADD)
            # transpose vT chunk -> v natural tiles (ones row kept intact)
            for ktl in range(4):
                kt = sc * 4 + ktl
                ptr = psAV.tile([128, 128], BF16, tag="av", name=f"ptr{kt}")
                nc.tensor.transpose(ptr[:], vT_sb[:, kt * 128:(kt + 1) * 128], ident[:])
                for g in range(2):
                    nc.vector.tensor_copy(
                        out=vaug[:, 0:64, g * KT + kt], in_=ptr[:, g * 64:(g + 1) * 64])

        # ================= phase 2: attention + out-proj =================
        for qc in range(SC):
            lo = qc * 512
            avT = avpool.tile([128, QT, 512], BF16, tag="avT", name=f"avT{qc}")
            for h in range(8):
                t, g = h % 4, h // 4
                ph = g * 64
                avp = psAV.tile([128, 512], F32, tag="av", name=f"avp{qc}_{h}")
                for kp in range(KT // 2):
                    sp = psA.tile([128, 1024], F32, tag="mm", name=f"sp{qc}_{h}_{kp}")
                    pe = ppool.tile([128, 1024], BF16, tag="pexp",
                                    name=f"pe{qc}_{h}_{kp}")
                    for j in range(2):
                        kt = 2 * kp + j
                        nc.tensor.matmul(
                            sp[:, j * 512:(j + 1) * 512],
                            kT_sb[ph:ph + 64, kt * 128:(kt + 1) * 128],
                            qT_sb[ph:ph + 64, t, lo:lo + 512],
                            start=True, stop=True)
                    nc.scalar.activation(
                        pe[:], sp[:], mybir.ActivationFunctionType.Exp,
                        scale=float(SCALE))
                    for j in range(2):
                        kt = 2 * kp + j
                        nc.tensor.matmul(
                            avp[0:65, :], vaug[:, :, g * KT + kt],
                            pe[:, j * 512:(j + 1) * 512],
                            start=(kt == 0), stop=(kt == KT - 1))
                # normalize: avT_norm = av[0:64] * (1 / av[64])
                den = npool.tile([1, 512], F32, tag="den", name=f"den{qc}_{h}")
                nc.vector.tensor_copy(out=den[:], in_=avp[64:65, :])
                linv = npool.tile([1, 512], F32, tag="linv", name=f"linv{qc}_{h}")
                nc.vector.reciprocal_approx_fast(out=linv[:], in_=den[:])
                lrep = npool.tile([64, 512], F32, tag="lrep", name=f"lrep{qc}_{h}")
                nc.gpsimd.partition_broadcast(out_ap=lrep[:], in_ap=linv[:])
                nc.vector.tensor_tensor(
                    out=avT[ph:ph + 64, t, :], in0=avp[0:64, :], in1=lrep[:],
                    op=MUL)
            # out projection for this q chunk
            for et in range(ET):
                yp = psY.tile([128, 512], F32, tag="y", name=f"yp{qc}_{et}")
                for t in range(QT):
                    nc.tensor.matmul(
                        yp[:], wo_sb[:, t, et * 128:(et + 1) * 128],
                        avT[:, t, :], start=(t == 0), stop=(t == QT - 1))
                ysb = ypool.tile([128, 512], F32, tag="ysb", name=f"ysb{qc}_{et}")
                nc.vector.tensor_copy(out=ysb[:], in_=yp[:])
                nc.sync.dma_start(out=yT[:, et, lo:lo + 512], in_=ysb[:])
    nc.compile()
    return nc


def _pm(a):
    """[E(=n*128), cols] -> partition-major [128, n, cols]."""
    a = np.ascontiguousarray(a)
    n = a.shape[0] // 128
    return np.ascontiguousarray(
        a.reshape(n, 128, a.shape[1]).transpose(1, 0, 2))


def _shard_inputs(x, Wq, bq, Wk, bk, Wv, bv, Wo, bo):
    """Build the 8 per-core input maps."""
    x = np.asarray(x, dtype=np.float32)
    in_maps = []
    for c in range(8):
        b, H = c // 2, c % 2
        heads = [8 * H + t for t in range(4)] + [8 * H + t + 4 for t in range(4)]
        # qT tile t holds (local head t -> partitions 0-63, local head t+4 -> 64-127)
        order = []
        for t in range(4):
            order.extend(range(heads[t] * 64, heads[t] * 64 + 64))
            order.extend(range(heads[t + 4] * 64, heads[t + 4] * 64 + 64))
        order = np.asarray(order)
        wq_p = _pm(np.asarray(Wq, np.float32)[:, order]).astype(BF16_NP)
        bq_p = np.ascontiguousarray(
            np.asarray(bq, np.float32)[order].reshape(4, 128).T)
        wo_p = _pm(np.asarray(Wo, np.float32)[order, :]).astype(BF16_NP)
        wk_s = _pm(np.asarray(Wk, np.float32)[:, H * 128:(H + 1) * 128]).astype(BF16_NP)
        wv_s = _pm(np.asarray(Wv, np.float32)[:, H * 128:(H + 1) * 128]).astype(BF16_NP)
        bk_s = np.ascontiguousarray(np.asarray(bk, np.float32)[H * 128:(H + 1) * 128]
                                    .reshape(128, 1))
        bv_s = np.ascontiguousarray(np.asarray(bv, np.float32)[H * 128:(H + 1) * 128]
                                    .reshape(128, 1))
        xT_b = _pm(np.ascontiguousarray(x[b].T)).astype(BF16_NP)
        in_maps.append({
            "xT": xT_b, "wq": wq_p, "wk": wk_s, "wv": wv_s, "wo": wo_p,
            "bqd": bq_p, "bkd": bk_s, "bvd": bv_s,
            "identd": np.eye(128, dtype=BF16_NP),
            "onesd": np.ones((128, 2 * KT), dtype=BF16_NP),
        })
    return in_maps


def kernel(x, Wq, bq, Wk, bk, Wv, bv, Wo, bo, _trace=False):
    if "nc" not in _NC_CACHE:
        _NC_CACHE["nc"] = build_nc()
    nc = _NC_CACHE["nc"]
    in_maps = _shard_inputs(x, Wq, bq, Wk, bk, Wv, bv, Wo, bo)
    res = run_bass_kernel_spmd(nc, in_maps, list(range(8)), trace=_trace)
    bo = np.asarray(bo, dtype=np.float32)
    out = np.empty((B, S, E), dtype=np.float32)
    for b in range(B):
        # yT dram layout [128, ET, S] -> [E, S]
        yTa = res.results[2 * b]["yT"] + res.results[2 * b + 1]["yT"]
        yE = yTa.transpose(1, 0, 2).reshape(E, S)
        out[b] = yE.T + bo
    if _trace:
        return out, res
    return out


# revision 9
# speedup vs baseline: 2.2326x; 1.1750x over previous
"""GQA self-attention kernel for Trainium2, sharded over 8 NeuronCores.

Problem: x[4, 2048, 1024], 16 heads / 4 KV groups / head_dim 64.
Sharding: batch (4) x head-half (2 KV groups each) = 8 cores.

All matmuls run in bf16 (PE 1 cycle/col; fp32 runs multi-pass). PSUM
accumulation stays fp32. exp runs on ACT over 1024-wide tiles spanning two
PSUM banks to amortize the per-instruction access latency. DMA issues are
spread across engine queues (sync/scalar/vector/gpsimd) because descriptor
generation (~0.6us per dma_start) serializes on a single engine otherwise.

Per-core dataflow (all "transposed world": features on partitions):
  xT[1024,2048] -> qT[512,2048], kT[128,2048], vT[128,2048] (PE matmuls)
  vT --PE transpose--> v_aug[seq,65] tiles (ones column appended)
  scores s[k,q] = kT_g^T(d,kpos) . qT_h(d,q)   (K=64 contraction)
  p = exp(s/8)  (ACT engine, PSUM f32 -> SBUF bf16, 1024 cols/instr)
  av[65,q] += v_aug^T p  (row 64 = softmax denominator)
  avT_norm = av[0:64] * recip_fast(av[64]) bcast (DVE + GpSimd broadcast)
  yT[e,q] = Wo_p^T . avT_norm  -> DRAM (f32)
Host: y[b] = (yT[2b] + yT[2b+1]).T + bo
"""

import sys
import numpy as np
import ml_dtypes

sys.path.insert(0, "/opt/trn_rl_repo")

from contextlib import ExitStack

import concourse.bass as bass
import concourse.bacc as bacc
import concourse.mybir as mybir
from concourse import tile
from concourse.bass_utils import run_bass_kernel_spmd

F32 = mybir.dt.float32
BF16 = mybir.dt.bfloat16
BF16_NP = ml_dtypes.bfloat16

B, S, E = 4, 2048, 1024
NUM_HEADS, NUM_GROUPS, D = 16, 4, 64
CQ = 512          # q cols per core (8 heads)
CK = 128          # kv cols per core (2 groups)
ET = E // 128     # 8 embed K-tiles
SC = S // 512     # 4 seq chunks of 512
KT = S // 128     # 16 key tiles of 128
QT = CQ // 128    # 4 qT partition tiles
SCALE = 1.0 / np.sqrt(np.float32(D))

_NC_CACHE = {}


def build_nc():
    nc = bacc.Bacc(None, target_bir_lowering=False)

    # DRAM layouts are partition-major ([128, blocks, cols]) so each logical
    # load/store is a single dma_start.
    xT = nc.dram_tensor("xT", [128, ET, S], BF16, kind="ExternalInput")
    wq = nc.dram_tensor("wq", [128, ET, CQ], BF16, kind="ExternalInput")
    wk = nc.dram_tensor("wk", [128, ET, CK], BF16, kind="ExternalInput")
    wv = nc.dram_tensor("wv", [128, ET, CK], BF16, kind="ExternalInput")
    wo = nc.dram_tensor("wo", [128, QT, E], BF16, kind="ExternalInput")
    bqd = nc.dram_tensor("bqd", [128, QT], F32, kind="ExternalInput")
    bkd = nc.dram_tensor("bkd", [128, 1], F32, kind="ExternalInput")
    bvd = nc.dram_tensor("bvd", [128, 1], F32, kind="ExternalInput")
    identd = nc.dram_tensor("identd", [128, 128], BF16, kind="ExternalInput")
    onesd = nc.dram_tensor("onesd", [128, 2 * KT], BF16, kind="ExternalInput")
    yT = nc.dram_tensor("yT", [128, ET, S], F32, kind="ExternalOutput")

    with tile.TileContext(nc) as tc, ExitStack() as ctx, \
            nc.allow_low_precision(reason="bf16 matmuls; rel-err budget 2e-2"):
        const = ctx.enter_context(tc.tile_pool(name="const", bufs=1))
        wpool = ctx.enter_context(tc.tile_pool(name="wpool", bufs=1))
        big = ctx.enter_context(tc.tile_pool(name="big", bufs=1))
        xpool = ctx.enter_context(tc.tile_pool(name="xpool", bufs=2))
        ppool = ctx.enter_context(tc.tile_pool(name="ppool", bufs=4))
        avpool = ctx.enter_context(tc.tile_pool(name="avpool", bufs=2))
        npool = ctx.enter_context(tc.tile_pool(name="npool", bufs=3))
        ypool = ctx.enter_context(tc.tile_pool(name="ypool", bufs=3))
        psA = ctx.enter_context(tc.tile_pool(name="psA", bufs=2, space="PSUM"))
        psAV = ctx.enter_context(tc.tile_pool(name="psAV", bufs=2, space="PSUM"))
        psY = ctx.enter_context(tc.tile_pool(name="psY", bufs=2, space="PSUM"))

        # ---- constants ----
        ident = const.tile([128, 128], BF16)
        nc.scalar.dma_start(out=ident[:], in_=identd[:, :])

        # ---- weights + biases (issue across engines to parallelize DGE) ----
        wq_sb = wpool.tile([128, ET, CQ], BF16)
        nc.sync.dma_start(out=wq_sb[:], in_=wq[:, :, :])
        wk_sb = wpool.tile([128, ET, CK], BF16)
        nc.gpsimd.dma_start(out=wk_sb[:], in_=wk[:, :, :])
        wv_sb = wpool.tile([128, ET, CK], BF16)
        nc.gpsimd.dma_start(out=wv_sb[:], in_=wv[:, :, :])
        wo_sb = wpool.tile([128, QT, E], BF16)
        nc.gpsimd.dma_start(out=wo_sb[:], in_=wo[:, :, :])
        bq_sb = wpool.tile([128, QT], F32)
        nc.scalar.dma_start(out=bq_sb[:], in_=bqd[:, :])
        bk_sb = wpool.tile([128, 1], F32)
        nc.scalar.dma_start(out=bk_sb[:], in_=bkd[:, :])
        bv_sb = wpool.tile([128, 1], F32)
        nc.scalar.dma_start(out=bv_sb[:], in_=bvd[:, :])

        # ---- persistent activations ----
        # one head per tile; complementary 64 rows stay zero so scores can
        # contract over K=128 (K=64 matmuls run at half rate on TRN2)
        qT_sb = big.tile([128, 8, S], BF16)       # 32KB/partition
        kT_sb = big.tile([128, S], BF16)          # 4KB
        vT_sb = big.tile([128, S], BF16)          # 4KB
        # v natural + ones row, laid out [part, 65, tile] so the ones row
        # loads in ONE dma  (tile idx = g*KT+kt; stationary AP vaug[:, :, i])
        vaug = big.tile([128, 65, 2 * KT], BF16)
        nc.scalar.dma_start(out=vaug[:, 64, :], in_=onesd[:, :])
        nc.vector.memset(qT_sb[64:128, 0:4, :], 0.0)
        nc.vector.memset(qT_sb[0:64, 4:8, :], 0.0)

        ADD = mybir.AluOpType.add
        MUL = mybir.AluOpType.mult

        # ================= phase 1: projections =================
        for sc in range(SC):
            lo = sc * 512
            xt = xpool.tile([128, ET, 512], BF16, tag="xt", name=f"xt{sc}")
            # spread the 8 block loads over 4 engine queues
            issuers = [nc.sync, nc.scalar, nc.gpsimd]
            for et in range(ET):
                issuers[et % 3].dma_start(
                    out=xt[:, et, :], in_=xT[:, et, lo:lo + 512])
            # qT
            for t in range(QT):
                pq = psA.tile([128, 1024], F32, tag="mm", name=f"pq{sc}_{t}")
                for et in range(ET):
                    nc.tensor.matmul(
                        pq[:, 0:512], wq_sb[:, et, t * 128:(t + 1) * 128],
                        xt[:, et, :], start=(et == 0), stop=(et == ET - 1))
                nc.vector.tensor_scalar(
                    out=qT_sb[0:64, t, lo:lo + 512], in0=pq[0:64, 0:512],
                    scalar1=bq_sb[0:64, t:t + 1], scalar2=None, op0=ADD)
                nc.vector.tensor_scalar(
                    out=qT_sb[64:128, t + 4, lo:lo + 512], in0=pq[64:128, 0:512],
                    scalar1=bq_sb[64:128, t:t + 1], scalar2=None, op0=ADD)
            # kT / vT
            pk = psA.tile([128, 1024], F32, tag="mm", name=f"pk{sc}")
            for et in range(ET):
                nc.tensor.matmul(pk[:, 0:512], wk_sb[:, et, :], xt[:, et, :],
                                 start=(et == 0), stop=(et == ET - 1))
            nc.vector.tensor_scalar(out=kT_sb[:, lo:lo + 512], in0=pk[:, 0:512],
                                    scalar1=bk_sb[:, 0:1], scalar2=None, op0=ADD)
            pv = psA.tile([128, 1024], F32, tag="mm", name=f"pv{sc}")
            for et in range(ET):
                nc.tensor.matmul(pv[:, 0:512], wv_sb[:, et, :], xt[:, et, :],
                                 start=(et == 0), stop=(et == ET - 1))
            nc.vector.tensor_scalar(out=vT_sb[:, lo:lo + 512], in0=pv[:, 0:512],
                                    scalar1=bv_sb[:, 0:1], scalar2=None, op0=ADD)
            # transpose vT chunk -> v natural tiles (ones row kept intact)
            for ktl in range(4):
                kt = sc * 4 + ktl
                ptr = psAV.tile([128, 128], BF16, tag="av", name=f"ptr{kt}")
                nc.tensor.transpose(ptr[:], vT_sb[:, kt * 128:(kt + 1) * 128], ident[:])
                for g in range(2):
                    nc.vector.tensor_copy(
                        out=vaug[:, 0:64, g * KT + kt], in_=ptr[:, g * 64:(g + 1) * 64])

        # ================= phase 2: attention + out-proj =================
        for qc in range(SC):
            lo = qc * 512
            avT = avpool.tile([128, QT, 512], BF16, tag="avT", name=f"avT{qc}")
            for h in range(8):
                t, g = h % 4, h // 4
                ph = g * 64
                avp = psAV.tile([128, 512], F32, tag="av", name=f"avp{qc}_{h}")
                for kp in range(KT // 2):
                    sp = psA.tile([128, 1024], F32, tag="mm", name=f"sp{qc}_{h}_{kp}")
                    pe = ppool.tile([128, 1024], BF16, tag="pexp",
                                    name=f"pe{qc}_{h}_{kp}")
                    for j in range(2):
                        kt = 2 * kp + j
                        nc.tensor.matmul(
                            sp[:, j * 512:(j + 1) * 512],
                            kT_sb[:, kt * 128:(kt + 1) * 128],
                            qT_sb[:, h, lo:lo + 512],
                            start=True, stop=True)
                    nc.scalar.activation(
                        pe[:], sp[:], mybir.ActivationFunctionType.Exp,
                        scale=float(SCALE))
                    for j in range(2):
                        kt = 2 * kp + j
                        nc.tensor.matmul(
                            avp[0:65, :], vaug[:, :, g * KT + kt],
                            pe[:, j * 512:(j + 1) * 512],
                            start=(kt == 0), stop=(kt == KT - 1))
                # normalize: avT_norm = av[0:64] * (1 / av[64])
                den = npool.tile([1, 512], F32, tag="den", name=f"den{qc}_{h}")
                nc.vector.tensor_copy(out=den[:], in_=avp[64:65, :])
                linv = npool.tile([1, 512], F32, tag="linv", name=f"linv{qc}_{h}")
                nc.vector.reciprocal_approx_fast(out=linv[:], in_=den[:])
                lrep = npool.tile([64, 512], F32, tag="lrep", name=f"lrep{qc}_{h}")
                nc.gpsimd.partition_broadcast(out_ap=lrep[:], in_ap=linv[:])
                nc.vector.tensor_tensor(
                    out=avT[ph:ph + 64, t, :], in0=avp[0:64, :], in1=lrep[:],
                    op=MUL)
            # out projection for this q chunk
            for et in range(ET):
                yp = psY.tile([128, 512], F32, tag="y", name=f"yp{qc}_{et}")
                for t in range(QT):
                    nc.tensor.matmul(
                        yp[:], wo_sb[:, t, et * 128:(et + 1) * 128],
                        avT[:, t, :], start=(t == 0), stop=(t == QT - 1))
                ysb = ypool.tile([128, 512], F32, tag="ysb", name=f"ysb{qc}_{et}")
                nc.vector.tensor_copy(out=ysb[:], in_=yp[:])
                nc.sync.dma_start(out=yT[:, et, lo:lo + 512], in_=ysb[:])
    nc.compile()
    return nc


def _pm(a):
    """[E(=n*128), cols] -> partition-major [128, n, cols]."""
    a = np.ascontiguousarray(a)
    n = a.shape[0] // 128
    return np.ascontiguousarray(
        a.reshape(n, 128, a.shape[1]).transpose(1, 0, 2))


def _shard_inputs(x, Wq, bq, Wk, bk, Wv, bv, Wo, bo):
    """Build the 8 per-core input maps."""
    x = np.asarray(x, dtype=np.float32)
    in_maps = []
    for c in range(8):
        b, H = c // 2, c % 2
        heads = [8 * H + t for t in range(4)] + [8 * H + t + 4 for t in range(4)]
        # qT tile t holds (local head t -> partitions 0-63, local head t+4 -> 64-127)
        order = []
        for t in range(4):
            order.extend(range(heads[t] * 64, heads[t] * 64 + 64))
            order.extend(range(heads[t + 4] * 64, heads[t + 4] * 64 + 64))
        order = np.asarray(order)
        wq_p = _pm(np.asarray(Wq, np.float32)[:, order]).astype(BF16_NP)
        bq_p = np.ascontiguousarray(
            np.asarray(bq, np.float32)[order].reshape(4, 128).T)
        wo_p = _pm(np.asarray(Wo, np.float32)[order, :]).astype(BF16_NP)
        wk_s = _pm(np.asarray(Wk, np.float32)[:, H * 128:(H + 1) * 128]).astype(BF16_NP)
        wv_s = _pm(np.asarray(Wv, np.float32)[:, H * 128:(H + 1) * 128]).astype(BF16_NP)
        bk_s = np.ascontiguousarray(np.asarray(bk, np.float32)[H * 128:(H + 1) * 128]
                                    .reshape(128, 1))
        bv_s = np.ascontiguousarray(np.asarray(bv, np.float32)[H * 128:(H + 1) * 128]
                                    .reshape(128, 1))
        xT_b = _pm(np.ascontiguousarray(x[b].T)).astype(BF16_NP)
        in_maps.append({
            "xT": xT_b, "wq": wq_p, "wk": wk_s, "wv": wv_s, "wo": wo_p,
            "bqd": bq_p, "bkd": bk_s, "bvd": bv_s,
            "identd": np.eye(128, dtype=BF16_NP),
            "onesd": np.ones((128, 2 * KT), dtype=BF16_NP),
        })
    return in_maps


def kernel(x, Wq, bq, Wk, bk, Wv, bv, Wo, bo, _trace=False):
    if "nc" not in _NC_CACHE:
        _NC_CACHE["nc"] = build_nc()
    nc = _NC_CACHE["nc"]
    in_maps = _shard_inputs(x, Wq, bq, Wk, bk, Wv, bv, Wo, bo)
    res = run_bass_kernel_spmd(nc, in_maps, list(range(8)), trace=_trace)
    bo = np.asarray(bo, dtype=np.float32)
    out = np.empty((B, S, E), dtype=np.float32)
    for b in range(B):
        # yT dram layout [128, ET, S] -> [E, S]
        yTa = res.results[2 * b]["yT"] + res.results[2 * b + 1]["yT"]
        yE = yTa.transpose(1, 0, 2).reshape(E, S)
        out[b] = yE.T + bo
    if _trace:
        return out, res
    return out
